# revision 1
# baseline (speedup 1.0000x reference)
"""Causal self-attention (B=4, S=2048, H=2048, 16 heads) on 8 Trainium2 NeuronCores.

Sharding: DP4 over batch x TP2 over heads. Core c handles batch c//2 and head
half c%2 (8 heads of 128 dims). fp16 matmul operands throughout (PSUM always
accumulates fp32); fp16 halves SBUF so the projection and attention phases
coexist and interleave, and halves DMA traffic. Per core:
  phase 1: V ([s,d] layout) and Q^T,K^T ([d,s] layout) projections, bounced to
           DRAM scratch (fp16). x^T loaded in two column-halves so the first
           projections start after 8MB, not 16MB.
  phase 2: per head, causal flash-style attention in the transposed layout
           (scores^T [k,q]): softmax denominators via ones-matmul, PV without
           transposes, exp batched 2 score tiles per ACTIVATE. Unnormalized
           ctx^T parked in SBUF; denominator strips collected in DRAM, one
           batched reciprocal per 2-head group (DRAM roundtrip to repack rows
           to partition 0), PE-broadcast, normalize into fp16. Emission
           interleaves attention(h) between later projections so PE gaps fill.
  phase 3: four pairwise fp16 AllGathers (one per 2-head group), launched as
           each group finishes so they overlap the remaining attention.
  phase 4: fp16 output projection from 16 ctx row-strips; the o-range is split
           across the pair via the per-core Wo slice (no program divergence),
           bias folded on host (bo_eff = bo + Wo @ bv; bv dropped from the V
           projection since softmax rows sum to 1). PSUM accumulation ordered
           chunk 0..3 so early chunks start before the last AllGather lands.
Host assembles out[b, :, o_half] = per-core out [s, o_half].
"""

import math
import sys

if "/opt/trn_rl_repo" not in sys.path:
    sys.path.insert(0, "/opt/trn_rl_repo")

import numpy as np

B, S, HID = 4, 2048, 2048
HEADS, D = 16, 128
HH = HEADS // 2          # heads per core
HHID = HH * D            # 1024, per-core head-span of hidden
KT = HID // 128          # 16 contraction tiles of 128
NB = S // 512            # 4 free-dim blocks of 512
N_CORES = 8
NCHUNK = 4               # ctx-exchange chunks (2 heads each)

_CACHED = {}


def _build_program():
    import concourse.tile as tile
    import concourse.mybir as mybir
    from concourse import bacc
    from concourse._compat import get_trn_type

    F32 = mybir.dt.float32
    F16 = mybir.dt.float16
    Exp = mybir.ActivationFunctionType.Exp
    Copy = mybir.ActivationFunctionType.Copy

    nc = bacc.Bacc(
        get_trn_type() or "TRN2",
        target_bir_lowering=False,
        debug=False,
        enable_asserts=False,
        num_devices=N_CORES,
    )

    def din(name, shape, dt=F16):
        return nc.dram_tensor(name, shape, dt, kind="ExternalInput").ap()

    xT = din("xT", [HID, S])          # x[b].T, fp16
    wqT = din("wqT", [HID, HHID])     # Wq.T columns for this core's heads
    wkT = din("wkT", [HID, HHID])
    wvT = din("wvT", [HID, HHID])
    woT = din("woT", [HID, HHID])     # Wo.T columns for this core's o-half
    bq = din("bq", [128, HH], F32)    # bq[h*128+p] at [p, h]
    bk = din("bk", [128, HH], F32)
    bo = din("bo", [1, HHID], F16)    # bo_eff slice for this core's o-half
    masks = din("masks", [4, 128, 512])
    out = nc.dram_tensor("out", [S, HHID], F32, kind="ExternalOutput").ap()

    inv_sqrt_d = float(1.0 / math.sqrt(D))

    with tile.TileContext(nc) as tc, \
         nc.allow_low_precision(reason="fp16 operand pipeline"):
        with tc.tile_pool(name="const", bufs=1) as constp, \
             tc.tile_pool(name="dram", bufs=1, space="DRAM") as dramp:
            # DRAM scratch (fp16 except denominator strips)
            qTd = dramp.tile([HHID, S], F16, tag="qTd")
            kTd = dramp.tile([HHID, S], F16, tag="kTd")
            vd = dramp.tile([S, HHID], F16, tag="vd")
            dden_d = [dramp.tile([8, 512], F32, tag=f"dden{c}",
                                 name=f"dden{c}") for c in range(NCHUNK)]
            rden_d = [dramp.tile([8, 512], F16, tag=f"rden{c}",
                                 name=f"rden{c}") for c in range(NCHUNK)]
            ctx_send = [dramp.tile([256, S], F16, tag=f"ctxs{c}",
                                   name=f"ctxs{c}") for c in range(NCHUNK)]
            ctx_recv = [dramp.tile([512, S], F16, tag=f"ctxr{c}",
                                   name=f"ctxr{c}") for c in range(NCHUNK)]

            # constants
            ones_col = constp.tile([128, 1], F16, tag="ones_col")
            nc.vector.memset(ones_col, 1.0)
            ones_row = constp.tile([1, 128], F16, tag="ones_row")
            nc.vector.memset(ones_row, 1.0)
            mask_t = []
            for r in range(4):
                mt = constp.tile([128, 512], F16, tag=f"mask{r}",
                                 name=f"mask{r}")
                nc.sync.dma_start(out=mt, in_=masks[r])
                mask_t.append(mt)
            bq_sb = constp.tile([128, HH], F32, tag="bq_sb")
            nc.sync.dma_start(out=bq_sb, in_=bq)
            bk_sb = constp.tile([128, HH], F32, tag="bk_sb")
            nc.sync.dma_start(out=bk_sb, in_=bk)
            bo_sb = constp.tile([1, HHID], F16, tag="bo_sb")
            nc.sync.dma_start(out=bo_sb, in_=bo)

            with tc.tile_pool(name="xk", bufs=2 * KT) as xp, \
                 tc.tile_pool(name="p1s", bufs=4) as sp, \
                 tc.tile_pool(name="p1w", bufs=16) as wp, \
                 tc.tile_pool(name="p1v", bufs=4) as vp, \
                 tc.tile_pool(name="p1wv", bufs=16) as wvp, \
                 tc.tile_pool(name="p2qk", bufs=2) as qkp, \
                 tc.tile_pool(name="p2v", bufs=18) as v4p, \
                 tc.tile_pool(name="p2et", bufs=4) as etp, \
                 tc.tile_pool(name="p2cu", bufs=10) as cup, \
                 tc.tile_pool(name="p2c", bufs=2) as cp, \
                 tc.tile_pool(name="p2d", bufs=2) as dp, \
                 tc.tile_pool(name="ps1", bufs=2, space="PSUM") as pp, \
                 tc.tile_pool(name="ps2s", bufs=2, space="PSUM") as pps, \
                 tc.tile_pool(name="ps2c", bufs=1, space="PSUM") as ppc, \
                 tc.tile_pool(name="ps2d", bufs=1, space="PSUM") as ppd:
                # x^T in two column-halves: [k][half] tiles of [128, 1024]
                xk = [[None, None] for _ in range(KT)]
                for half in range(2):
                    for k in range(KT):
                        t = xp.tile([128, 1024], F16, tag="xk",
                                    name=f"xk{k}_{half}")
                        nc.sync.dma_start(
                            out=t,
                            in_=xT[k * 128:(k + 1) * 128,
                                   half * 1024:(half + 1) * 1024])
                        xk[k][half] = t

                def xslice(k, lo, size):
                    half, off = lo // 1024, lo % 1024
                    return xk[k][half][:, off:off + size]

                def v_proj(g):
                    wvt = []
                    for k in range(KT):
                        w = wvp.tile([128, 512], F16, tag="wv",
                                     name=f"wv{g}_{k}")
                        nc.sync.dma_start(
                            out=w,
                            in_=wvT[k * 128:(k + 1) * 128,
                                    g * 512:(g + 1) * 512])
                        wvt.append(w)
                    for m in range(KT):
                        ps = pp.tile([128, 512], F32, tag="ps1",
                                     name=f"psv{g}_{m}")
                        for k in range(KT):
                            nc.tensor.matmul(
                                ps, xslice(k, m * 128, 128), wvt[k],
                                start=(k == 0), stop=(k == KT - 1))
                        vsb = vp.tile([128, 512], F16, tag="vout",
                                      name=f"v{g}_{m}")
                        nc.vector.tensor_copy(out=vsb, in_=ps)
                        nc.sync.dma_start(
                            out=vd[m * 128:(m + 1) * 128,
                                   g * 512:(g + 1) * 512], in_=vsb)

                def qk_proj(h):
                    for wT, dst, bias_sb, pname in (
                        (wqT, qTd, bq_sb, "q"),
                        (wkT, kTd, bk_sb, "k"),
                    ):
                        wt = []
                        for k in range(KT):
                            w = wp.tile([128, 128], F16, tag="w",
                                        name=f"w{pname}{h}_{k}")
                            nc.sync.dma_start(
                                out=w,
                                in_=wT[k * 128:(k + 1) * 128,
                                       h * 128:(h + 1) * 128])
                            wt.append(w)
                        for n in range(NB):
                            ps = pp.tile([128, 512], F32, tag="ps1",
                                         name=f"ps{pname}{h}_{n}")
                            for k in range(KT):
                                nc.tensor.matmul(
                                    ps, wt[k], xslice(k, n * 512, 512),
                                    start=(k == 0), stop=(k == KT - 1))
                            osb = sp.tile([128, 512], F16, tag="projout",
                                          name=f"o{pname}{h}_{n}")
                            nc.vector.tensor_scalar_add(
                                osb, ps, bias_sb[:, h:h + 1])
                            nc.sync.dma_start(
                                out=dst[h * 128:(h + 1) * 128,
                                        n * 512:(n + 1) * 512], in_=osb)

                v4 = [None, None]
                ctx_u = {}

                def load_v4(g):
                    tiles = []
                    for k in range(KT):
                        t = v4p.tile([128, 512], F16, tag="v4",
                                     name=f"v4_{g}_{k}")
                        nc.sync.dma_start(
                            out=t,
                            in_=vd[k * 128:(k + 1) * 128,
                                   g * 512:(g + 1) * 512])
                        tiles.append(t)
                    v4[g] = tiles

                def attention(h):
                    g, sub = h // 4, h % 4
                    qh = qkp.tile([128, S], F16, tag="qh", name=f"qh{h}")
                    nc.sync.dma_start(out=qh,
                                      in_=qTd[h * 128:(h + 1) * 128, :])
                    kh = qkp.tile([128, S], F16, tag="kh", name=f"kh{h}")
                    nc.sync.dma_start(out=kh,
                                      in_=kTd[h * 128:(h + 1) * 128, :])
                    for qb in range(NB):
                        kept = min(4 * qb + 4, KT)
                        ctx_ps = ppc.tile([128, 512], F32, tag="ctxps",
                                          name=f"cps{h}_{qb}")
                        den_ps = ppd.tile([1, 512], F32, tag="denps",
                                          name=f"dps{h}_{qb}")
                        for kt0 in range(0, kept, 2):
                            sps = pps.tile([128, 1024], F32, tag="sps",
                                           name=f"sps{h}_{qb}_{kt0}")
                            for i in range(2):
                                nc.tensor.matmul(
                                    sps[:, i * 512:(i + 1) * 512],
                                    kh[:, (kt0 + i) * 128:(kt0 + i + 1) * 128],
                                    qh[:, qb * 512:(qb + 1) * 512],
                                    start=True, stop=True)
                            et = etp.tile([128, 1024], F16, tag="et",
                                          name=f"et{h}_{qb}_{kt0}")
                            nc.scalar.activation(out=et, in_=sps, func=Exp,
                                                 scale=inv_sqrt_d)
                            for i in range(2):
                                kt = kt0 + i
                                ets = et[:, i * 512:(i + 1) * 512]
                                r = kt - 4 * qb
                                if r >= 0:
                                    nc.vector.tensor_mul(ets, ets, mask_t[r])
                                nc.tensor.matmul(
                                    den_ps, ones_col, ets,
                                    start=(kt == 0), stop=(kt == kept - 1))
                                nc.tensor.matmul(
                                    ctx_ps,
                                    v4[g][kt][:, sub * 128:(sub + 1) * 128],
                                    ets,
                                    start=(kt == 0), stop=(kt == kept - 1))
                        row = (h % 2) * 4 + qb
                        den_sb = dp.tile([1, 512], F32, tag="densb",
                                         name=f"den{h}_{qb}", bufs=3)
                        nc.vector.tensor_copy(out=den_sb, in_=den_ps)
                        nc.sync.dma_start(
                            out=dden_d[h // 2][row:row + 1, :], in_=den_sb)
                        cu = cup.tile([128, 512], F32, tag="cu",
                                      name=f"cu{h}_{qb}")
                        nc.scalar.activation(out=cu, in_=ctx_ps, func=Copy)
                        ctx_u[(h, qb)] = cu
                    if h % 2 == 1:
                        c = h // 2
                        dpack = dp.tile([8, 512], F32, tag="dpack",
                                        name=f"dpack{c}", bufs=2)
                        nc.sync.dma_start(out=dpack, in_=dden_d[c])
                        rpack = dp.tile([8, 512], F16, tag="rpack",
                                        name=f"rpack{c}", bufs=2)
                        nc.vector.reciprocal(out=rpack, in_=dpack)
                        nc.sync.dma_start(out=rden_d[c], in_=rpack)
                        rstrip = dp.tile([1, 8, 512], F16, tag="rstrip",
                                         name=f"rstrip{c}", bufs=2)
                        nc.sync.dma_start(
                            out=rstrip,
                            in_=rden_d[c].rearrange("(o r) c -> o r c", o=1))
                        for hh in (h - 1, h):
                            ct16 = cp.tile([128, S], F16, tag="ctxh",
                                           name=f"ctxh{hh}")
                            for qb in range(NB):
                                row = (hh % 2) * 4 + qb
                                dbc_ps = pp.tile([128, 512], F32, tag="ps1",
                                                 name=f"dbc{hh}_{qb}")
                                nc.tensor.matmul(
                                    dbc_ps, ones_row, rstrip[:, row, :],
                                    start=True, stop=True)
                                nc.vector.tensor_mul(
                                    ct16[:, qb * 512:(qb + 1) * 512],
                                    ctx_u[(hh, qb)], dbc_ps)
                            nc.sync.dma_start(
                                out=ctx_send[c][(hh % 2) * 128:
                                                (hh % 2) * 128 + 128, :],
                                in_=ct16)
                        nc.gpsimd.collective_compute(
                            "AllGather",
                            mybir.AluOpType.bypass,
                            replica_groups=[[0, 1], [2, 3], [4, 5], [6, 7]],
                            ins=[ctx_send[c].opt()],
                            outs=[ctx_recv[c].opt()],
                        )

                # interleaved emission: projections feed attention per head
                qk_proj(0)
                qk_proj(1)
                v_proj(0)
                load_v4(0)
                qk_proj(2)
                attention(0)
                qk_proj(3)
                attention(1)
                v_proj(1)
                qk_proj(4)
                attention(2)
                qk_proj(5)
                attention(3)
                load_v4(1)
                qk_proj(6)
                attention(4)
                qk_proj(7)
                attention(5)
                attention(6)
                attention(7)

            # ---------------- phase 4: output projection ----------------
            korder = []
            for c in range(NCHUNK):
                korder += [(c, 0, 2 * c), (c, 128, 2 * c + 1),
                           (c, 256, 8 + 2 * c), (c, 384, 8 + 2 * c + 1)]
            with tc.tile_pool(name="p4wo", bufs=32) as wop, \
                 tc.tile_pool(name="p4ct", bufs=16) as ctp, \
                 tc.tile_pool(name="p4o", bufs=3) as op_, \
                 tc.tile_pool(name="p4b", bufs=1) as bp4, \
                 tc.tile_pool(name="ps4", bufs=4, space="PSUM") as pp4:
                # broadcast bo across partitions via ones outer product
                bo_bc = bp4.tile([128, HHID], F32, tag="bo_bc")
                for n in range(HHID // 512):
                    bps = pp4.tile([128, 512], F32, tag="ps4", name=f"bps{n}")
                    nc.tensor.matmul(bps, ones_row,
                                     bo_sb[:, n * 512:(n + 1) * 512],
                                     start=True, stop=True)
                    nc.vector.tensor_copy(out=bo_bc[:, n * 512:(n + 1) * 512],
                                          in_=bps)
                wo = {}
                ct = []
                for ki, (c, off, gk) in enumerate(korder):
                    t = ctp.tile([128, S], F16, tag="ct", name=f"ct{gk}")
                    nc.sync.dma_start(out=t, in_=ctx_recv[c][off:off + 128, :])
                    ct.append(t)
                    for n in range(HHID // 512):
                        w = wop.tile([128, 512], F16, tag="wo",
                                     name=f"wo{gk}_{n}")
                        nc.sync.dma_start(
                            out=w,
                            in_=woT[gk * 128:(gk + 1) * 128,
                                    n * 512:(n + 1) * 512])
                        wo[(ki, n)] = w
                for m in range(S // 128):
                    osb = op_.tile([128, HHID], F32, tag="osb", name=f"osb{m}")
                    for n in range(HHID // 512):
                        ps = pp4.tile([128, 512], F32, tag="ps4",
                                      name=f"ps4_{m}_{n}")
                        for ki in range(KT):
                            nc.tensor.matmul(
                                ps, ct[ki][:, m * 128:(m + 1) * 128],
                                wo[(ki, n)],
                                start=(ki == 0), stop=(ki == KT - 1))
                        nc.vector.tensor_add(
                            osb[:, n * 512:(n + 1) * 512], ps,
                            bo_bc[:, n * 512:(n + 1) * 512])
                    nc.sync.dma_start(out=out[m * 128:(m + 1) * 128, :],
                                      in_=osb)

    nc.compile()
    return nc


def _get_nc():
    if "nc" not in _CACHED:
        _CACHED["nc"] = _build_program()
    return _CACHED["nc"]


def _make_masks():
    i = np.arange(128)[:, None]
    j = np.arange(512)[None, :]
    return np.stack(
        [((j - i) >= 128 * r).astype(np.float16) for r in range(4)], axis=0)


def _make_in_maps(inputs):
    x = np.ascontiguousarray(np.asarray(inputs["x"], dtype=np.float32))
    Wq = np.asarray(inputs["Wq"], dtype=np.float32)
    Wk = np.asarray(inputs["Wk"], dtype=np.float32)
    Wv = np.asarray(inputs["Wv"], dtype=np.float32)
    Wo = np.asarray(inputs["Wo"], dtype=np.float32)
    bq = np.asarray(inputs["bq"], dtype=np.float32)
    bk = np.asarray(inputs["bk"], dtype=np.float32)
    bv = np.asarray(inputs["bv"], dtype=np.float32)
    bo = np.asarray(inputs["bo"], dtype=np.float32)

    bo_eff = bo + Wo @ bv
    masks = _make_masks()
    WqT = np.ascontiguousarray(Wq.T)
    WkT = np.ascontiguousarray(Wk.T)
    WvT = np.ascontiguousarray(Wv.T)
    WoT = np.ascontiguousarray(Wo.T)

    in_maps = []
    for c in range(N_CORES):
        b, hf = c // 2, c % 2
        sl = slice(hf * HHID, (hf + 1) * HHID)
        in_maps.append({
            "xT": np.ascontiguousarray(x[b].T).astype(np.float16),
            "wqT": np.ascontiguousarray(WqT[:, sl]).astype(np.float16),
            "wkT": np.ascontiguousarray(WkT[:, sl]).astype(np.float16),
            "wvT": np.ascontiguousarray(WvT[:, sl]).astype(np.float16),
            "woT": np.ascontiguousarray(WoT[:, sl]).astype(np.float16),
            "bq": np.ascontiguousarray(bq[sl].reshape(HH, 128).T),
            "bk": np.ascontiguousarray(bk[sl].reshape(HH, 128).T),
            "bo": bo_eff[sl].reshape(1, HHID).astype(np.float16),
            "masks": masks,
        })
    return in_maps


def kernel(**inputs):
    from concourse.bass_utils import run_bass_kernel_spmd

    in_maps = _make_in_maps(inputs)
    nc = _get_nc()
    res = run_bass_kernel_spmd(nc, in_maps, list(range(N_CORES)))

    out = np.empty((B, S, HID), dtype=np.float32)
    for c in range(N_CORES):
        b, hf = c // 2, c % 2
        out[b, :, hf * HHID:(hf + 1) * HHID] = res.results[c]["out"]
    return out



# revision 8
# speedup vs baseline: 1.1544x; 1.1544x over previous
"""Causal self-attention (B=4, S=2048, H=2048, 16 heads) on 8 Trainium2 NeuronCores.

Sharding: DP4 over batch x TP2 over heads. Core c handles batch c//2 and head
half c%2 (8 heads of 128 dims). fp16 matmul operands throughout (PSUM always
accumulates fp32). Per core:
  phase 1: V projection kept resident in SBUF ([s,d] layout); Q^T,K^T
           projections ([d,s] layout) bounced to DRAM scratch (fp16). x^T
           loaded in two column-halves interleaved with the first projection
           weights so the first matmuls start early.
  phase 2: per head, causal flash-style attention in the transposed layout
           (scores^T [k,q]): exp batched 2 score tiles per ACTIVATE (scalar),
           mask + softmax-denominator accumulation on the vector engine (f32
           accumulator), one [1,512] ones-matmul per (head, q-block) for the
           partition reduction, immediate reciprocal + PE-broadcast +
           normalize into fp16 ct16. Emission interleaves attention(h)
           between later projections so PE gaps fill.
  phase 3: eight pairwise fp16 AllGathers (one per head), launched as each
           head finishes so they overlap the remaining attention.
  phase 4: fp16 output projection from 16 ctx row-strips; the o-range is
           split across the pair via the per-core Wo slice (no program
           divergence), bias folded on host (bo_eff = bo + Wo @ bv; bv
           dropped from the V projection since softmax rows sum to 1). PSUM
           accumulation ordered head 0..7 so early strips start before the
           last AllGather lands.
Host assembles out[b, :, o_half] = per-core out [s, o_half].
"""

import math
import sys

if "/opt/trn_rl_repo" not in sys.path:
    sys.path.insert(0, "/opt/trn_rl_repo")

import numpy as np

B, S, HID = 4, 2048, 2048
HEADS, D = 16, 128
HH = HEADS // 2          # heads per core
HHID = HH * D            # 1024, per-core head-span of hidden
KT = HID // 128          # 16 contraction tiles of 128
NB = S // 512            # 4 free-dim blocks of 512
N_CORES = 8

_CACHED = {}


def _build_program():
    import concourse.tile as tile
    import concourse.mybir as mybir
    from concourse import bacc
    from concourse._compat import get_trn_type

    F32 = mybir.dt.float32
    F16 = mybir.dt.float16
    Exp = mybir.ActivationFunctionType.Exp

    nc = bacc.Bacc(
        get_trn_type() or "TRN2",
        target_bir_lowering=False,
        debug=False,
        enable_asserts=False,
        num_devices=N_CORES,
    )

    def din(name, shape, dt=F16):
        return nc.dram_tensor(name, shape, dt, kind="ExternalInput").ap()

    xT = din("xT", [HID, S])          # x[b].T, fp16
    wqT = din("wqT", [HID, HHID])     # Wq.T columns for this core's heads
    wkT = din("wkT", [HID, HHID])
    wvT = din("wvT", [HID, HHID])
    woT = din("woT", [HID, HHID])     # Wo.T columns for this core's o-half
    bq = din("bq", [128, HH], F32)    # bq[h*128+p] at [p, h]
    bk = din("bk", [128, HH], F32)
    bo = din("bo", [1, HHID], F16)    # bo_eff slice for this core's o-half
    masks = din("masks", [4, 128, 512])
    out = nc.dram_tensor("out", [S, HHID], F32, kind="ExternalOutput").ap()

    inv_sqrt_d = float(1.0 / math.sqrt(D))

    with tile.TileContext(nc) as tc, \
         nc.allow_low_precision(reason="fp16 operand pipeline"):
        with tc.tile_pool(name="const", bufs=1) as constp, \
             tc.tile_pool(name="dram", bufs=1, space="DRAM") as dramp:
            # DRAM scratch (fp16)
            qTd = dramp.tile([HHID, S], F16, tag="qTd")
            kTd = dramp.tile([HHID, S], F16, tag="kTd")
            ctx_send = [dramp.tile([128, S], F16, tag=f"ctxs{h}",
                                   name=f"ctxs{h}") for h in range(HH)]
            ctx_recv = [dramp.tile([256, S], F16, tag=f"ctxr{h}",
                                   name=f"ctxr{h}") for h in range(HH)]

            # constants
            ones_col = constp.tile([128, 1], F16, tag="ones_col")
            nc.vector.memset(ones_col, 1.0)
            ones_row = constp.tile([1, 128], F16, tag="ones_row")
            nc.vector.memset(ones_row, 1.0)
            mask_t = []
            for r in range(4):
                mt = constp.tile([128, 512], F16, tag=f"mask{r}",
                                 name=f"mask{r}")
                nc.sync.dma_start(out=mt, in_=masks[r])
                mask_t.append(mt)
            bq_sb = constp.tile([128, HH], F32, tag="bq_sb")
            nc.sync.dma_start(out=bq_sb, in_=bq)
            bk_sb = constp.tile([128, HH], F32, tag="bk_sb")
            nc.sync.dma_start(out=bk_sb, in_=bk)
            bo_sb = constp.tile([1, HHID], F16, tag="bo_sb")
            nc.sync.dma_start(out=bo_sb, in_=bo)

            with tc.tile_pool(name="xk", bufs=2 * KT) as xp, \
                 tc.tile_pool(name="p1s", bufs=4) as sp, \
                 tc.tile_pool(name="p1w", bufs=48) as wp, \
                 tc.tile_pool(name="p1wv", bufs=16) as wvp, \
                 tc.tile_pool(name="p2qk", bufs=4) as qkp, \
                 tc.tile_pool(name="p2v", bufs=34) as v4p, \
                 tc.tile_pool(name="p2et", bufs=6) as etp, \
                 tc.tile_pool(name="p2acc", bufs=3) as accp, \
                 tc.tile_pool(name="p2a16", bufs=2) as acc16p, \
                 tc.tile_pool(name="p2c", bufs=2) as cp, \
                 tc.tile_pool(name="p2r", bufs=4) as rqp, \
                 tc.tile_pool(name="p2db", bufs=3) as dbp, \
                 tc.tile_pool(name="ps1", bufs=2, space="PSUM") as pp, \
                 tc.tile_pool(name="ps2s", bufs=2, space="PSUM") as pps, \
                 tc.tile_pool(name="ps2c", bufs=2, space="PSUM") as ppc:
                # x^T in two column-halves: [k][half] tiles of [128, 1024]
                xk = [[None, None] for _ in range(KT)]

                def load_xk(half):
                    for k in range(KT):
                        t = xp.tile([128, 1024], F16, tag="xk",
                                    name=f"xk{k}_{half}")
                        nc.sync.dma_start(
                            out=t,
                            in_=xT[k * 128:(k + 1) * 128,
                                   half * 1024:(half + 1) * 1024])
                        xk[k][half] = t

                def xslice(k, lo, size):
                    half, off = lo // 1024, lo % 1024
                    return xk[k][half][:, off:off + size]

                v4 = [[None] * KT, [None] * KT]

                def v_proj(g):
                    wvt = []
                    for k in range(KT):
                        w = wvp.tile([128, 512], F16, tag="wv",
                                     name=f"wv{g}_{k}")
                        nc.sync.dma_start(
                            out=w,
                            in_=wvT[k * 128:(k + 1) * 128,
                                    g * 512:(g + 1) * 512])
                        wvt.append(w)
                    for m in range(KT):
                        ps = pp.tile([128, 512], F32, tag="ps1",
                                     name=f"psv{g}_{m}")
                        for k in range(KT):
                            nc.tensor.matmul(
                                ps, xslice(k, m * 128, 128), wvt[k],
                                start=(k == 0), stop=(k == KT - 1))
                        vsb = v4p.tile([128, 512], F16, tag="v4",
                                       name=f"v4_{g}_{m}")
                        nc.vector.tensor_copy(out=vsb, in_=ps)
                        v4[g][m] = vsb

                def qk_weights(h):
                    wts = {}
                    for wT, pname in ((wqT, "q"), (wkT, "k")):
                        wt = []
                        for k in range(KT):
                            w = wp.tile([128, 128], F16, tag="w",
                                        name=f"w{pname}{h}_{k}")
                            nc.sync.dma_start(
                                out=w,
                                in_=wT[k * 128:(k + 1) * 128,
                                       h * 128:(h + 1) * 128])
                            wt.append(w)
                        wts[pname] = wt
                    return wts

                def qk_proj(h, wts=None):
                    if wts is None:
                        wts = qk_weights(h)
                    for dst, bias_sb, pname in (
                        (qTd, bq_sb, "q"),
                        (kTd, bk_sb, "k"),
                    ):
                        wt = wts[pname]
                        for n in range(NB):
                            ps = pp.tile([128, 512], F32, tag="ps1",
                                         name=f"ps{pname}{h}_{n}")
                            for k in range(KT):
                                nc.tensor.matmul(
                                    ps, wt[k], xslice(k, n * 512, 512),
                                    start=(k == 0), stop=(k == KT - 1))
                            osb = sp.tile([128, 512], F16, tag="projout",
                                          name=f"o{pname}{h}_{n}")
                            nc.vector.tensor_scalar_add(
                                osb, ps, bias_sb[:, h:h + 1])
                            nc.sync.dma_start(
                                out=dst[h * 128:(h + 1) * 128,
                                        n * 512:(n + 1) * 512], in_=osb)

                def attention(h):
                    g, sub = h // 4, h % 4
                    qh = qkp.tile([128, S], F16, tag="qh", name=f"qh{h}")
                    nc.sync.dma_start(out=qh,
                                      in_=qTd[h * 128:(h + 1) * 128, :])
                    kh = qkp.tile([128, S], F16, tag="kh", name=f"kh{h}")
                    nc.sync.dma_start(out=kh,
                                      in_=kTd[h * 128:(h + 1) * 128, :])
                    ct16 = cp.tile([128, S], F16, tag="ctxh",
                                   name=f"ctxh{h}")
                    for qb in range(NB):
                        kept = 4 * qb + 4
                        ctx_ps = ppc.tile([128, 512], F32, tag="ctxps",
                                          name=f"cps{h}_{qb}")
                        acc = accp.tile([128, 512], F32, tag="acc",
                                        name=f"acc{h}_{qb}")
                        acc16 = acc16p.tile([128, 512], F16, tag="acc16",
                                            name=f"acc16_{h}_{qb}")
                        for kt0 in range(0, kept, 2):
                            sps = pps.tile([128, 1024], F32, tag="sps",
                                           name=f"sps{h}_{qb}_{kt0}")
                            for i in range(2):
                                nc.tensor.matmul(
                                    sps[:, i * 512:(i + 1) * 512],
                                    kh[:, (kt0 + i) * 128:(kt0 + i + 1) * 128],
                                    qh[:, qb * 512:(qb + 1) * 512],
                                    start=True, stop=True)
                            et = etp.tile([128, 1024], F16, tag="et",
                                          name=f"et{h}_{qb}_{kt0}")
                            nc.scalar.activation(out=et, in_=sps, func=Exp,
                                                 scale=inv_sqrt_d)
                            for i in range(2):
                                kt = kt0 + i
                                ets = et[:, i * 512:(i + 1) * 512]
                                r = kt - 4 * qb
                                if r >= 0:
                                    nc.vector.tensor_mul(ets, ets, mask_t[r])
                                nc.tensor.matmul(
                                    ctx_ps,
                                    v4[g][kt][:, sub * 128:(sub + 1) * 128],
                                    ets,
                                    start=(kt == 0), stop=(kt == kept - 1))
                                if kt == 0:
                                    nc.vector.tensor_copy(out=acc, in_=ets)
                                elif kt == kept - 1:
                                    nc.vector.tensor_add(acc16, acc, ets)
                                else:
                                    nc.vector.tensor_add(acc, acc, ets)
                        den_ps = pp.tile([1, 512], F32, tag="ps1",
                                         name=f"dps{h}_{qb}")
                        nc.tensor.matmul(den_ps, ones_col, acc16,
                                         start=True, stop=True)
                        rq = rqp.tile([1, 512], F16, tag="rq",
                                      name=f"rq{h}_{qb}")
                        nc.vector.reciprocal(out=rq, in_=den_ps)
                        dbc_ps = pp.tile([128, 512], F32, tag="ps1",
                                         name=f"dbc{h}_{qb}")
                        nc.tensor.matmul(dbc_ps, ones_row, rq,
                                         start=True, stop=True)
                        dbc_sb = dbp.tile([128, 512], F32, tag="dbc",
                                          name=f"dbcs{h}_{qb}")
                        nc.vector.tensor_copy(out=dbc_sb, in_=dbc_ps)
                        nc.vector.tensor_mul(
                            ct16[:, qb * 512:(qb + 1) * 512],
                            ctx_ps, dbc_sb)
                    nc.sync.dma_start(out=ctx_send[h], in_=ct16)
                    nc.gpsimd.collective_compute(
                        "AllGather",
                        mybir.AluOpType.bypass,
                        replica_groups=[[0, 1], [2, 3], [4, 5], [6, 7]],
                        ins=[ctx_send[h].opt()],
                        outs=[ctx_recv[h].opt()],
                    )

                # interleaved emission: projections feed attention per head
                load_xk(0)
                wts0 = qk_weights(0)
                load_xk(1)
                qk_proj(0, wts0)
                qk_proj(1)
                v_proj(0)
                qk_proj(2)
                attention(0)
                qk_proj(3)
                attention(1)
                v_proj(1)
                qk_proj(4)
                attention(2)
                qk_proj(5)
                attention(3)
                qk_proj(6)
                attention(4)
                qk_proj(7)
                attention(5)
                attention(6)
                attention(7)

            # ---------------- phase 4: output projection ----------------
            korder = []
            for h in range(HH):
                korder += [(h, 0, h), (h, 128, 8 + h)]
            with tc.tile_pool(name="p4wo", bufs=32) as wop, \
                 tc.tile_pool(name="p4ct", bufs=16) as ctp, \
                 tc.tile_pool(name="p4o", bufs=3) as op_, \
                 tc.tile_pool(name="p4b", bufs=1) as bp4, \
                 tc.tile_pool(name="ps4", bufs=4, space="PSUM") as pp4:
                # broadcast bo across partitions via ones outer product
                bo_bc = bp4.tile([128, HHID], F32, tag="bo_bc")
                for n in range(HHID // 512):
                    bps = pp4.tile([128, 512], F32, tag="ps4", name=f"bps{n}")
                    nc.tensor.matmul(bps, ones_row,
                                     bo_sb[:, n * 512:(n + 1) * 512],
                                     start=True, stop=True)
                    nc.vector.tensor_copy(out=bo_bc[:, n * 512:(n + 1) * 512],
                                          in_=bps)
                wo = {}
                ct = []
                for ki, (c, off, gk) in enumerate(korder):
                    t = ctp.tile([128, S], F16, tag="ct", name=f"ct{gk}")
                    nc.sync.dma_start(out=t, in_=ctx_recv[c][off:off + 128, :])
                    ct.append(t)
                    for n in range(HHID // 512):
                        w = wop.tile([128, 512], F16, tag="wo",
                                     name=f"wo{gk}_{n}")
                        nc.sync.dma_start(
                            out=w,
                            in_=woT[gk * 128:(gk + 1) * 128,
                                    n * 512:(n + 1) * 512])
                        wo[(ki, n)] = w
                for m in range(S // 128):
                    osb = op_.tile([128, HHID], F32, tag="osb", name=f"osb{m}")
                    for n in range(HHID // 512):
                        ps = pp4.tile([128, 512], F32, tag="ps4",
                                      name=f"ps4_{m}_{n}")
                        for ki in range(KT):
                            nc.tensor.matmul(
                                ps, ct[ki][:, m * 128:(m + 1) * 128],
                                wo[(ki, n)],
                                start=(ki == 0), stop=(ki == KT - 1))
                        nc.vector.tensor_add(
                            osb[:, n * 512:(n + 1) * 512], ps,
                            bo_bc[:, n * 512:(n + 1) * 512])
                    nc.sync.dma_start(out=out[m * 128:(m + 1) * 128, :],
                                      in_=osb)

    nc.compile()
    return nc


def _get_nc():
    if "nc" not in _CACHED:
        _CACHED["nc"] = _build_program()
    return _CACHED["nc"]


def _make_masks():
    i = np.arange(128)[:, None]
    j = np.arange(512)[None, :]
    return np.stack(
        [((j - i) >= 128 * r).astype(np.float16) for r in range(4)], axis=0)


def _make_in_maps(inputs):
    x = np.ascontiguousarray(np.asarray(inputs["x"], dtype=np.float32))
    Wq = np.asarray(inputs["Wq"], dtype=np.float32)
    Wk = np.asarray(inputs["Wk"], dtype=np.float32)
    Wv = np.asarray(inputs["Wv"], dtype=np.float32)
    Wo = np.asarray(inputs["Wo"], dtype=np.float32)
    bq = np.asarray(inputs["bq"], dtype=np.float32)
    bk = np.asarray(inputs["bk"], dtype=np.float32)
    bv = np.asarray(inputs["bv"], dtype=np.float32)
    bo = np.asarray(inputs["bo"], dtype=np.float32)

    bo_eff = bo + Wo @ bv
    masks = _make_masks()
    WqT = np.ascontiguousarray(Wq.T)
    WkT = np.ascontiguousarray(Wk.T)
    WvT = np.ascontiguousarray(Wv.T)
    WoT = np.ascontiguousarray(Wo.T)

    in_maps = []
    for c in range(N_CORES):
        b, hf = c // 2, c % 2
        sl = slice(hf * HHID, (hf + 1) * HHID)
        in_maps.append({
            "xT": np.ascontiguousarray(x[b].T).astype(np.float16),
            "wqT": np.ascontiguousarray(WqT[:, sl]).astype(np.float16),
            "wkT": np.ascontiguousarray(WkT[:, sl]).astype(np.float16),
            "wvT": np.ascontiguousarray(WvT[:, sl]).astype(np.float16),
            "woT": np.ascontiguousarray(WoT[:, sl]).astype(np.float16),
            "bq": np.ascontiguousarray(bq[sl].reshape(HH, 128).T),
            "bk": np.ascontiguousarray(bk[sl].reshape(HH, 128).T),
            "bo": bo_eff[sl].reshape(1, HHID).astype(np.float16),
            "masks": masks,
        })
    return in_maps


def kernel(**inputs):
    from concourse.bass_utils import run_bass_kernel_spmd

    in_maps = _make_in_maps(inputs)
    nc = _get_nc()
    res = run_bass_kernel_spmd(nc, in_maps, list(range(N_CORES)))

    out = np.empty((B, S, HID), dtype=np.float32)
    for c in range(N_CORES):
        b, hf = c // 2, c % 2
        out[b, :, hf * HHID:(hf + 1) * HHID] = res.results[c]["out"]
    return out


# revision 17
# speedup vs baseline: 1.2573x; 1.0891x over previous
"""Causal self-attention (B=4, S=2048, H=2048, 16 heads) on 8 Trainium2 NeuronCores.

Sharding: DP4 over batch x TP2 over heads. Core c handles batch c//2 and head
half c%2 (8 heads of 128 dims). fp16 matmul operands throughout (PSUM always
accumulates fp32). Per core:
  phase 1: V projection kept resident in SBUF ([s,d] layout); Q^T,K^T
           projections ([d,s] layout) bounced to DRAM scratch (fp16). x^T
           loaded in quarter-tiles interleaved with the first projection
           weights so the first matmuls start early.
  phase 2: per head, causal flash-style attention in the transposed layout
           (scores^T [k,q]): exp batched 2 score tiles per ACTIVATE (scalar),
           paired masks, softmax-denominator accumulated on the vector engine
           (f32 accumulator, fp16 pair-sums), one [1,512] ones-matmul per
           (head, q-block) for the partition reduction, then
           reciprocal_approx_fast + gpsimd partition_broadcast + normalize
           into fp16 ct16. Emission interleaves attention(h) between later
           projections so PE gaps fill.
  phase 3: eight pairwise fp16 AllGathers (one per head), launched as each
           head finishes so they overlap the remaining attention.
  phase 4: fp16 output projection from 16 ctx row-strips loaded as per-m
           [128,128] slices; emitted early (before the last attention heads)
           so strip DMAs and early accumulation overlap the attention tail.
           o-range split across the pair via the per-core Wo slice; bias
           folded on host (bo_eff = bo + Wo @ bv).
Host assembles out[b, :, o_half] = per-core out [s, o_half].
"""

import contextlib
import math
import sys

if "/opt/trn_rl_repo" not in sys.path:
    sys.path.insert(0, "/opt/trn_rl_repo")

import numpy as np

B, S, HID = 4, 2048, 2048
HEADS, D = 16, 128
HH = HEADS // 2          # heads per core
HHID = HH * D            # 1024, per-core head-span of hidden
KT = HID // 128          # 16 contraction tiles of 128
NB = S // 512            # 4 free-dim blocks of 512
NQ = 4                   # x^T column quarters of 512
N_CORES = 8

_CACHED = {}


def _build_program():
    import concourse.tile as tile
    import concourse.mybir as mybir
    from concourse import bacc
    from concourse._compat import get_trn_type

    F32 = mybir.dt.float32
    F16 = mybir.dt.float16
    Exp = mybir.ActivationFunctionType.Exp

    nc = bacc.Bacc(
        get_trn_type() or "TRN2",
        target_bir_lowering=False,
        debug=False,
        enable_asserts=False,
        num_devices=N_CORES,
    )

    def din(name, shape, dt=F16):
        return nc.dram_tensor(name, shape, dt, kind="ExternalInput").ap()

    xT = din("xT", [HID, S])          # x[b].T, fp16
    wqT = din("wqT", [HID, HHID])     # Wq.T columns for this core's heads
    wkT = din("wkT", [HID, HHID])
    wvT = din("wvT", [HID, HHID])
    woT = din("woT", [HID, HHID])     # Wo.T columns for this core's o-half
    bq = din("bq", [128, HH], F32)    # bq[h*128+p] at [p, h]
    bk = din("bk", [128, HH], F32)
    bo = din("bo", [1, HHID], F16)    # bo_eff slice for this core's o-half
    masks = din("masks", [2, 128, 1024])   # paired causal masks
    out = nc.dram_tensor("out", [S, HHID], F32, kind="ExternalOutput").ap()

    inv_sqrt_d = float(1.0 / math.sqrt(D))

    with tile.TileContext(nc) as tc, \
         nc.allow_low_precision(reason="fp16 operand pipeline"):
        with tc.tile_pool(name="const", bufs=1) as constp, \
             tc.tile_pool(name="dram", bufs=1, space="DRAM") as dramp:
            # DRAM scratch (fp16)
            qTd = dramp.tile([HHID, S], F16, tag="qTd")
            kTd = dramp.tile([HHID, S], F16, tag="kTd")
            ctx_send = [dramp.tile([128, S], F16, tag=f"ctxs{h}",
                                   name=f"ctxs{h}") for h in range(HH)]
            ctx_recv = [dramp.tile([256, S], F16, tag=f"ctxr{h}",
                                   name=f"ctxr{h}") for h in range(HH)]

            # constants
            ones_col = constp.tile([128, 1], F16, tag="ones_col")
            nc.vector.memset(ones_col, 1.0)
            ones_row = constp.tile([1, 128], F16, tag="ones_row")
            nc.vector.memset(ones_row, 1.0)
            mask2 = []
            for r in range(2):
                mt = constp.tile([128, 1024], F16, tag=f"mask{r}",
                                 name=f"mask{r}")
                nc.sync.dma_start(out=mt, in_=masks[r])
                mask2.append(mt)
            bq_sb = constp.tile([128, HH], F32, tag="bq_sb")
            nc.sync.dma_start(out=bq_sb, in_=bq)
            bk_sb = constp.tile([128, HH], F32, tag="bk_sb")
            nc.sync.dma_start(out=bk_sb, in_=bk)
            bo_sb = constp.tile([1, HHID], F16, tag="bo_sb")
            nc.sync.dma_start(out=bo_sb, in_=bo)

            with contextlib.ExitStack() as p2stack:
                ec = p2stack.enter_context
                qkp = ec(tc.tile_pool(name="p2qk", bufs=4))
                v4p = ec(tc.tile_pool(name="p2v", bufs=34))
                etp = ec(tc.tile_pool(name="p2et", bufs=6))
                accp = ec(tc.tile_pool(name="p2acc", bufs=3))
                acc16p = ec(tc.tile_pool(name="p2a16", bufs=2))
                pair16p = ec(tc.tile_pool(name="p2p16", bufs=2))
                cp = ec(tc.tile_pool(name="p2c", bufs=2))
                rqp = ec(tc.tile_pool(name="p2r", bufs=2))
                dbp = ec(tc.tile_pool(name="p2db", bufs=2))
                pps = ec(tc.tile_pool(name="ps2s", bufs=2, space="PSUM"))
                ppc = ec(tc.tile_pool(name="ps2c", bufs=2, space="PSUM"))

                v4 = [[None] * KT, [None] * KT]
                xk = [[None] * NQ for _ in range(KT)]

                def attention(h):
                    g, sub = h // 4, h % 4
                    qh = qkp.tile([128, S], F16, tag="qh", name=f"qh{h}")
                    nc.sync.dma_start(out=qh,
                                      in_=qTd[h * 128:(h + 1) * 128, :])
                    kh = qkp.tile([128, S], F16, tag="kh", name=f"kh{h}")
                    nc.sync.dma_start(out=kh,
                                      in_=kTd[h * 128:(h + 1) * 128, :])
                    ct16 = cp.tile([128, S], F16, tag="ctxh",
                                   name=f"ctxh{h}")
                    for qb in range(NB):
                        kept = 4 * qb + 4
                        ctx_ps = ppc.tile([128, 512], F32, tag="ctxps",
                                          name=f"cps{h}_{qb}")
                        acc = accp.tile([128, 512], F32, tag="acc",
                                        name=f"acc{h}_{qb}")
                        acc16 = acc16p.tile([128, 512], F16, tag="acc16",
                                            name=f"acc16_{h}_{qb}")
                        for kt0 in range(0, kept, 2):
                            sps = pps.tile([128, 1024], F32, tag="sps",
                                           name=f"sps{h}_{qb}_{kt0}")
                            for i in range(2):
                                nc.tensor.matmul(
                                    sps[:, i * 512:(i + 1) * 512],
                                    kh[:, (kt0 + i) * 128:(kt0 + i + 1) * 128],
                                    qh[:, qb * 512:(qb + 1) * 512],
                                    start=True, stop=True)
                            et = etp.tile([128, 1024], F16, tag="et",
                                          name=f"et{h}_{qb}_{kt0}")
                            nc.scalar.activation(out=et, in_=sps, func=Exp,
                                                 scale=inv_sqrt_d)
                            # paired causal mask on the diagonal tiles
                            if kt0 == 4 * qb:
                                nc.vector.tensor_mul(et, et, mask2[0])
                            elif kt0 == 4 * qb + 2:
                                nc.vector.tensor_mul(et, et, mask2[1])
                            for i in range(2):
                                kt = kt0 + i
                                nc.tensor.matmul(
                                    ctx_ps,
                                    v4[g][kt][:, sub * 128:(sub + 1) * 128],
                                    et[:, i * 512:(i + 1) * 512],
                                    start=(kt == 0), stop=(kt == kept - 1))
                            # denominator accumulation (vector engine)
                            last = kt0 + 2 >= kept
                            if kt0 == 0:
                                dst = acc16 if last else acc
                                nc.vector.tensor_add(
                                    dst, et[:, 0:512], et[:, 512:1024])
                            else:
                                p16 = pair16p.tile([128, 512], F16,
                                                   tag="p16",
                                                   name=f"p16_{h}_{qb}_{kt0}")
                                nc.vector.tensor_add(
                                    p16, et[:, 0:512], et[:, 512:1024])
                                dst = acc16 if last else acc
                                nc.vector.tensor_add(dst, acc, p16)
                        den_ps = ppc.tile([1, 512], F32, tag="ctxps",
                                          name=f"dps{h}_{qb}")
                        nc.tensor.matmul(den_ps, ones_col, acc16,
                                         start=True, stop=True)
                        rq = rqp.tile([1, 512], F32, tag="rq",
                                      name=f"rq{h}_{qb}")
                        nc.vector.reciprocal_approx_fast(out=rq, in_=den_ps)
                        dbc = dbp.tile([128, 512], F32, tag="dbc",
                                       name=f"dbc{h}_{qb}")
                        nc.gpsimd.partition_broadcast(dbc, rq)
                        nc.vector.tensor_mul(
                            ct16[:, qb * 512:(qb + 1) * 512],
                            ctx_ps, dbc)
                    nc.sync.dma_start(out=ctx_send[h], in_=ct16)
                    nc.gpsimd.collective_compute(
                        "AllGather",
                        mybir.AluOpType.bypass,
                        replica_groups=[[0, 1], [2, 3], [4, 5], [6, 7]],
                        ins=[ctx_send[h].opt()],
                        outs=[ctx_recv[h].opt()],
                    )

                with contextlib.ExitStack() as p1stack:
                    ec1 = p1stack.enter_context
                    xp = ec1(tc.tile_pool(name="xk", bufs=NQ * KT))
                    sp = ec1(tc.tile_pool(name="p1s", bufs=4))
                    wp = ec1(tc.tile_pool(name="p1w", bufs=48))
                    wvp = ec1(tc.tile_pool(name="p1wv", bufs=16))
                    pp = ec1(tc.tile_pool(name="ps1", bufs=2, space="PSUM"))

                    def load_xq(q):
                        for k in range(KT):
                            t = xp.tile([128, 512], F16, tag="xk",
                                        name=f"xk{k}_{q}")
                            nc.sync.dma_start(
                                out=t,
                                in_=xT[k * 128:(k + 1) * 128,
                                       q * 512:(q + 1) * 512])
                            xk[k][q] = t

                    def xslice(k, lo, size):
                        q, off = lo // 512, lo % 512
                        assert off + size <= 512
                        return xk[k][q][:, off:off + size]

                    def v_proj(g):
                        wvt = []
                        for k in range(KT):
                            w = wvp.tile([128, 512], F16, tag="wv",
                                         name=f"wv{g}_{k}")
                            nc.sync.dma_start(
                                out=w,
                                in_=wvT[k * 128:(k + 1) * 128,
                                        g * 512:(g + 1) * 512])
                            wvt.append(w)
                        for m in range(KT):
                            ps = pp.tile([128, 512], F32, tag="ps1",
                                         name=f"psv{g}_{m}")
                            for k in range(KT):
                                nc.tensor.matmul(
                                    ps, xslice(k, m * 128, 128), wvt[k],
                                    start=(k == 0), stop=(k == KT - 1))
                            vsb = v4p.tile([128, 512], F16, tag="v4",
                                           name=f"v4_{g}_{m}")
                            nc.vector.tensor_copy(out=vsb, in_=ps)
                            v4[g][m] = vsb

                    def qk_weights(h):
                        wts = {}
                        for wT, pname in ((wqT, "q"), (wkT, "k")):
                            wt = []
                            for k in range(KT):
                                w = wp.tile([128, 128], F16, tag="w",
                                            name=f"w{pname}{h}_{k}")
                                nc.sync.dma_start(
                                    out=w,
                                    in_=wT[k * 128:(k + 1) * 128,
                                           h * 128:(h + 1) * 128])
                                wt.append(w)
                            wts[pname] = wt
                        return wts

                    def qk_proj(h, wts=None):
                        if wts is None:
                            wts = qk_weights(h)
                        for dst, bias_sb, pname in (
                            (qTd, bq_sb, "q"),
                            (kTd, bk_sb, "k"),
                        ):
                            wt = wts[pname]
                            for n in range(NB):
                                ps = pp.tile([128, 512], F32, tag="ps1",
                                             name=f"ps{pname}{h}_{n}")
                                for k in range(KT):
                                    nc.tensor.matmul(
                                        ps, wt[k], xslice(k, n * 512, 512),
                                        start=(k == 0), stop=(k == KT - 1))
                                osb = sp.tile([128, 512], F16, tag="projout",
                                              name=f"o{pname}{h}_{n}")
                                nc.vector.tensor_scalar_add(
                                    osb, ps, bias_sb[:, h:h + 1])
                                nc.sync.dma_start(
                                    out=dst[h * 128:(h + 1) * 128,
                                            n * 512:(n + 1) * 512], in_=osb)

                    # startup: first-needed tiles first
                    load_xq(0)
                    wts0 = qk_weights(0)
                    load_xq(1)
                    load_xq(2)
                    load_xq(3)
                    qk_proj(0, wts0)
                    qk_proj(1)
                    v_proj(0)
                    qk_proj(2)
                    attention(0)
                    qk_proj(3)
                    attention(1)
                    v_proj(1)
                    qk_proj(4)
                    attention(2)
                    qk_proj(5)
                    attention(3)
                    qk_proj(6)
                    attention(4)
                    qk_proj(7)
                    attention(5)

                # phase-1 pools closed: xk/weights SBUF and proj PSUM free.
                # phase 4 opens here so its DMAs/accumulation overlap the
                # attention tail (heads 6,7).
                korder = []
                for h in range(HH):
                    korder += [(h, 0, h), (h, 128, 8 + h)]
                with contextlib.ExitStack() as p4stack:
                    ec4 = p4stack.enter_context
                    wop = ec4(tc.tile_pool(name="p4wo", bufs=32))
                    ctp = ec4(tc.tile_pool(name="p4ct", bufs=48))
                    op_ = ec4(tc.tile_pool(name="p4o", bufs=2))
                    bp4 = ec4(tc.tile_pool(name="p4b", bufs=1))
                    pp4 = ec4(tc.tile_pool(name="ps4", bufs=2, space="PSUM"))
                    attention(6)

                    bo_bc = bp4.tile([128, HHID], F32, tag="bo_bc")
                    for n in range(HHID // 512):
                        bps = pp4.tile([128, 512], F32, tag="ps4",
                                       name=f"bps{n}")
                        nc.tensor.matmul(bps, ones_row,
                                         bo_sb[:, n * 512:(n + 1) * 512],
                                         start=True, stop=True)
                        nc.vector.tensor_copy(
                            out=bo_bc[:, n * 512:(n + 1) * 512], in_=bps)
                    wo = {}
                    for ki, (c, off, gk) in enumerate(korder):
                        for n in range(HHID // 512):
                            w = wop.tile([128, 512], F16, tag="wo",
                                         name=f"wo{gk}_{n}")
                            nc.sync.dma_start(
                                out=w,
                                in_=woT[gk * 128:(gk + 1) * 128,
                                        n * 512:(n + 1) * 512])
                            wo[(ki, n)] = w

                    attention(7)

                    # per-m ct strip slices: [128,128] tiles, 16 per m
                    def load_ct_m(m):
                        tiles = []
                        for ki, (c, off, gk) in enumerate(korder):
                            t = ctp.tile([128, 128], F16, tag="ctm",
                                         name=f"ctm{m}_{ki}")
                            nc.sync.dma_start(
                                out=t,
                                in_=ctx_recv[c][off:off + 128,
                                                m * 128:(m + 1) * 128])
                            tiles.append(t)
                        return tiles

                    ct_m = {m: load_ct_m(m) for m in range(2)}

                    for m in range(S // 128):
                        if m + 2 < S // 128:
                            ct_m[m + 2] = load_ct_m(m + 2)
                        tiles = ct_m.pop(m)
                        osb = op_.tile([128, HHID], F32, tag="osb",
                                       name=f"osb{m}")
                        for n in range(HHID // 512):
                            ps = pp4.tile([128, 512], F32, tag="ps4",
                                          name=f"ps4_{m}_{n}")
                            for ki in range(KT):
                                nc.tensor.matmul(
                                    ps, tiles[ki], wo[(ki, n)],
                                    start=(ki == 0), stop=(ki == KT - 1))
                            nc.vector.tensor_add(
                                osb[:, n * 512:(n + 1) * 512], ps,
                                bo_bc[:, n * 512:(n + 1) * 512])
                        nc.sync.dma_start(out=out[m * 128:(m + 1) * 128, :],
                                          in_=osb)

    nc.compile()
    return nc


def _get_nc():
    if "nc" not in _CACHED:
        _CACHED["nc"] = _build_program()
    return _CACHED["nc"]


def _make_masks():
    i = np.arange(128)[:, None]
    j = np.arange(1024)[None, :]
    m4 = [((j - i) >= 128 * r).astype(np.float16) for r in range(4)]
    # paired: [mask0 | mask1], [mask2 | mask3] over 512-col halves
    p0 = np.concatenate([m4[0][:, :512], m4[1][:, :512]], axis=1)
    p1 = np.concatenate([m4[2][:, :512], m4[3][:, :512]], axis=1)
    return np.stack([p0, p1], axis=0)


def _make_in_maps(inputs):
    x = np.ascontiguousarray(np.asarray(inputs["x"], dtype=np.float32))
    Wq = np.asarray(inputs["Wq"], dtype=np.float32)
    Wk = np.asarray(inputs["Wk"], dtype=np.float32)
    Wv = np.asarray(inputs["Wv"], dtype=np.float32)
    Wo = np.asarray(inputs["Wo"], dtype=np.float32)
    bq = np.asarray(inputs["bq"], dtype=np.float32)
    bk = np.asarray(inputs["bk"], dtype=np.float32)
    bv = np.asarray(inputs["bv"], dtype=np.float32)
    bo = np.asarray(inputs["bo"], dtype=np.float32)

    bo_eff = bo + Wo @ bv
    masks = _make_masks()
    WqT = np.ascontiguousarray(Wq.T)
    WkT = np.ascontiguousarray(Wk.T)
    WvT = np.ascontiguousarray(Wv.T)
    WoT = np.ascontiguousarray(Wo.T)

    in_maps = []
    for c in range(N_CORES):
        b, hf = c // 2, c % 2
        sl = slice(hf * HHID, (hf + 1) * HHID)
        in_maps.append({
            "xT": np.ascontiguousarray(x[b].T).astype(np.float16),
            "wqT": np.ascontiguousarray(WqT[:, sl]).astype(np.float16),
            "wkT": np.ascontiguousarray(WkT[:, sl]).astype(np.float16),
            "wvT": np.ascontiguousarray(WvT[:, sl]).astype(np.float16),
            "woT": np.ascontiguousarray(WoT[:, sl]).astype(np.float16),
            "bq": np.ascontiguousarray(bq[sl].reshape(HH, 128).T),
            "bk": np.ascontiguousarray(bk[sl].reshape(HH, 128).T),
            "bo": bo_eff[sl].reshape(1, HHID).astype(np.float16),
            "masks": masks,
        })
    return in_maps


def kernel(**inputs):
    from concourse.bass_utils import run_bass_kernel_spmd

    in_maps = _make_in_maps(inputs)
    nc = _get_nc()
    res = run_bass_kernel_spmd(nc, in_maps, list(range(N_CORES)))

    out = np.empty((B, S, HID), dtype=np.float32)
    for c in range(N_CORES):
        b, hf = c // 2, c % 2
        out[b, :, hf * HHID:(hf + 1) * HHID] = res.results[c]["out"]
    return out


# revision 34
# speedup vs baseline: 1.4117x; 1.1228x over previous
"""Causal self-attention (B=4, S=2048, H=2048, 16 heads) on 8 Trainium2 NeuronCores.

Sharding: DP4 over batch x TP2 over heads. Core c handles batch c//2 and head
half c%2 (8 heads of 128 dims). fp16 matmul operands throughout (PSUM always
accumulates fp32). Per core:
  phase 1: V projection kept resident in SBUF ([s,d] layout); Q^T,K^T
           projections ([d,s] layout) bounced to DRAM scratch (fp16). x^T
           loaded in quarter-tiles interleaved with the first projection
           weights so the first matmuls start early.
  phase 2: per head, causal flash-style attention in the transposed layout
           (scores^T [k,q]): exp batched 2 score tiles per ACTIVATE (scalar),
           paired masks, softmax-denominator accumulated on the vector engine
           (f32 accumulator, fp16 pair-sums), one [1,512] ones-matmul per
           (head, q-block) for the partition reduction, then
           reciprocal_approx_fast + gpsimd partition_broadcast + normalize
           into fp16 ct16. Emission interleaves attention(h) between later
           projections so PE gaps fill.
  phase 3: eight pairwise fp16 AllGathers (one per head), launched as each
           head finishes so they overlap the remaining attention.
  phase 4: fp16 output projection from 16 ctx row-strips loaded as per-m
           [128,128] slices; emitted early (before the last attention heads)
           so strip DMAs and early accumulation overlap the attention tail.
           o-range split across the pair via the per-core Wo slice; bias
           folded on host (bo_eff = bo + Wo @ bv).
Host assembles out[b, :, o_half] = per-core out [s, o_half].
"""

import contextlib
import math
import sys

if "/opt/trn_rl_repo" not in sys.path:
    sys.path.insert(0, "/opt/trn_rl_repo")

import numpy as np

B, S, HID = 4, 2048, 2048
HEADS, D = 16, 128
HH = HEADS // 2          # heads per core
HHID = HH * D            # 1024, per-core head-span of hidden
KT = HID // 128          # 16 contraction tiles of 128
NB = S // 512            # 4 free-dim blocks of 512
NQ = 4                   # x^T column quarters of 512
N_CORES = 8

_CACHED = {}


def _build_program():
    import concourse.tile as tile
    import concourse.mybir as mybir
    from concourse import bacc
    from concourse._compat import get_trn_type

    F32 = mybir.dt.float32
    F16 = mybir.dt.float16
    Exp = mybir.ActivationFunctionType.Exp

    nc = bacc.Bacc(
        get_trn_type() or "TRN2",
        target_bir_lowering=False,
        debug=False,
        enable_asserts=False,
        num_devices=N_CORES,
    )

    def din(name, shape, dt=F16):
        return nc.dram_tensor(name, shape, dt, kind="ExternalInput").ap()

    xT = din("xT", [HID, S])          # x[b].T, fp16
    # per-head packed q/k weights: [:, h*2048 + k*128 + d] = WqT[k*128+p, ...]
    wqT = din("wqT", [128, HH * HID])
    wkT = din("wkT", [128, HH * HID])
    wvT = din("wvT", [HID, HHID])
    woT = din("woT", [HID, HHID])     # Wo.T columns for this core's o-half
    bq = din("bq", [128, HH], F32)    # bq[h*128+p] at [p, h]
    bk = din("bk", [128, HH], F32)
    bo = din("bo", [1, HHID], F16)    # bo_eff slice for this core's o-half
    masks = din("masks", [2, 128, 1024])   # paired causal masks
    out = nc.dram_tensor("out", [S, HHID], F32, kind="ExternalOutput").ap()

    inv_sqrt_d = float(1.0 / math.sqrt(D))

    with tile.TileContext(nc) as tc, \
         nc.allow_low_precision(reason="fp16 operand pipeline"):
        with tc.tile_pool(name="const", bufs=1) as constp, \
             tc.tile_pool(name="dram", bufs=1, space="DRAM") as dramp:
            # DRAM scratch (fp16)
            qTd = dramp.tile([HHID, S], F16, tag="qTd")
            kTd = dramp.tile([HHID, S], F16, tag="kTd")
            # column-split exchange: a = q-cols 0:1536 (after qb2), b = rest
            SA, SB = 1536, 512
            send_a = [dramp.tile([128, SA], F16, tag=f"ctxsa{h}",
                                 name=f"ctxsa{h}") for h in range(HH)]
            send_b = [dramp.tile([128, SB], F16, tag=f"ctxsb{h}",
                                 name=f"ctxsb{h}") for h in range(HH)]
            recv_a = [dramp.tile([256, SA], F16, tag=f"ctxra{h}",
                                 name=f"ctxra{h}") for h in range(HH)]
            recv_b = [dramp.tile([256, SB], F16, tag=f"ctxrb{h}",
                                 name=f"ctxrb{h}") for h in range(HH)]

            # constants
            ones_col = constp.tile([128, 1], F16, tag="ones_col")
            nc.vector.memset(ones_col, 1.0)
            ones_row = constp.tile([1, 128], F16, tag="ones_row")
            nc.vector.memset(ones_row, 1.0)
            mask2 = []
            for r in range(2):
                mt = constp.tile([128, 1024], F16, tag=f"mask{r}",
                                 name=f"mask{r}")
                nc.sync.dma_start(out=mt, in_=masks[r])
                mask2.append(mt)
            bq_sb = constp.tile([128, HH], F32, tag="bq_sb")
            nc.sync.dma_start(out=bq_sb, in_=bq)
            bk_sb = constp.tile([128, HH], F32, tag="bk_sb")
            nc.sync.dma_start(out=bk_sb, in_=bk)
            bo_sb = constp.tile([1, HHID], F16, tag="bo_sb")
            nc.sync.dma_start(out=bo_sb, in_=bo)

            with contextlib.ExitStack() as p2stack:
                ec = p2stack.enter_context
                qkp = ec(tc.tile_pool(name="p2qk", bufs=4))
                v4p = ec(tc.tile_pool(name="p2v", bufs=32))
                etp = ec(tc.tile_pool(name="p2et", bufs=5))
                accp = ec(tc.tile_pool(name="p2acc", bufs=2))
                acc16p = ec(tc.tile_pool(name="p2a16", bufs=2))
                pair16p = ec(tc.tile_pool(name="p2p16", bufs=1))
                cp = ec(tc.tile_pool(name="p2c", bufs=2))
                rqp = ec(tc.tile_pool(name="p2r", bufs=1))
                dbp = ec(tc.tile_pool(name="p2db", bufs=2))
                pps = ec(tc.tile_pool(name="ps2s", bufs=2, space="PSUM"))
                ppc = ec(tc.tile_pool(name="ps2c", bufs=2, space="PSUM"))

                v4 = [[None] * KT, [None] * KT]
                xk = [[None] * NQ for _ in range(KT)]

                def qk_fetch(h):
                    qh = qkp.tile([128, S], F16, tag="qh", name=f"qh{h}")
                    nc.sync.dma_start(out=qh,
                                      in_=qTd[h * 128:(h + 1) * 128, :])
                    kh = qkp.tile([128, S], F16, tag="kh", name=f"kh{h}")
                    nc.sync.dma_start(out=kh,
                                      in_=kTd[h * 128:(h + 1) * 128, :])
                    return qh, kh

                def attention(h, qk=None):
                    g, sub = h // 4, h % 4
                    qh, kh = qk if qk is not None else qk_fetch(h)
                    ct16 = cp.tile([128, S], F16, tag="ctxh",
                                   name=f"ctxh{h}")
                    for qb in range(NB):
                        kept = 4 * qb + 4
                        ctx_ps = ppc.tile([128, 512], F32, tag="ctxps",
                                          name=f"cps{h}_{qb}")
                        acc = accp.tile([128, 512], F32, tag="acc",
                                        name=f"acc{h}_{qb}")
                        acc16 = acc16p.tile([128, 512], F16, tag="acc16",
                                            name=f"acc16_{h}_{qb}")
                        for kt0 in range(0, kept, 2):
                            sps = pps.tile([128, 1024], F32, tag="sps",
                                           name=f"sps{h}_{qb}_{kt0}")
                            for i in range(2):
                                nc.tensor.matmul(
                                    sps[:, i * 512:(i + 1) * 512],
                                    kh[:, (kt0 + i) * 128:(kt0 + i + 1) * 128],
                                    qh[:, qb * 512:(qb + 1) * 512],
                                    start=True, stop=True)
                            et = etp.tile([128, 1024], F16, tag="et",
                                          name=f"et{h}_{qb}_{kt0}")
                            nc.scalar.activation(out=et, in_=sps, func=Exp,
                                                 scale=inv_sqrt_d)
                            # paired causal mask on the diagonal tiles
                            if kt0 == 4 * qb:
                                nc.vector.tensor_mul(et, et, mask2[0])
                            elif kt0 == 4 * qb + 2:
                                nc.vector.tensor_mul(et, et, mask2[1])
                            for i in range(2):
                                kt = kt0 + i
                                nc.tensor.matmul(
                                    ctx_ps,
                                    v4[g][kt][:, sub * 128:(sub + 1) * 128],
                                    et[:, i * 512:(i + 1) * 512],
                                    start=(kt == 0), stop=(kt == kept - 1))
                            # denominator accumulation (vector engine)
                            last = kt0 + 2 >= kept
                            if kt0 == 0:
                                dst = acc16 if last else acc
                                nc.vector.tensor_add(
                                    dst, et[:, 0:512], et[:, 512:1024])
                            else:
                                p16 = pair16p.tile([128, 512], F16,
                                                   tag="p16",
                                                   name=f"p16_{h}_{qb}_{kt0}")
                                nc.vector.tensor_add(
                                    p16, et[:, 0:512], et[:, 512:1024])
                                dst = acc16 if last else acc
                                nc.vector.tensor_add(dst, acc, p16)
                        den_ps = ppc.tile([1, 512], F32, tag="ctxps",
                                          name=f"dps{h}_{qb}")
                        nc.tensor.matmul(den_ps, ones_col, acc16,
                                         start=True, stop=True)
                        rq = rqp.tile([1, 512], F32, tag="rq",
                                      name=f"rq{h}_{qb}")
                        nc.vector.reciprocal_approx_fast(out=rq, in_=den_ps)
                        dbc = dbp.tile([128, 512], F32, tag="dbc",
                                       name=f"dbc{h}_{qb}")
                        nc.gpsimd.partition_broadcast(dbc, rq)
                        nc.vector.tensor_mul(
                            ct16[:, qb * 512:(qb + 1) * 512],
                            ctx_ps, dbc)
                        if qb == 2:
                            nc.sync.dma_start(out=send_a[h],
                                              in_=ct16[:, 0:SA])
                            nc.gpsimd.collective_compute(
                                "AllGather",
                                mybir.AluOpType.bypass,
                                replica_groups=[[0, 1], [2, 3],
                                                [4, 5], [6, 7]],
                                ins=[send_a[h].opt()],
                                outs=[recv_a[h].opt()],
                            )
                    nc.sync.dma_start(out=send_b[h], in_=ct16[:, SA:S])
                    nc.gpsimd.collective_compute(
                        "AllGather",
                        mybir.AluOpType.bypass,
                        replica_groups=[[0, 1], [2, 3], [4, 5], [6, 7]],
                        ins=[send_b[h].opt()],
                        outs=[recv_b[h].opt()],
                    )

                with contextlib.ExitStack() as p1stack:
                    ec1 = p1stack.enter_context
                    xp = ec1(tc.tile_pool(name="xk", bufs=NQ * KT))
                    sp = ec1(tc.tile_pool(name="p1s", bufs=4))
                    wp = ec1(tc.tile_pool(name="p1w", bufs=3))
                    wvp = ec1(tc.tile_pool(name="p1wv", bufs=16))
                    pp = ec1(tc.tile_pool(name="ps1", bufs=2, space="PSUM"))

                    def load_xq(q):
                        for k in range(KT):
                            t = xp.tile([128, 512], F16, tag="xk",
                                        name=f"xk{k}_{q}")
                            nc.sync.dma_start(
                                out=t,
                                in_=xT[k * 128:(k + 1) * 128,
                                       q * 512:(q + 1) * 512])
                            xk[k][q] = t

                    def xslice(k, lo, size):
                        q, off = lo // 512, lo % 512
                        assert off + size <= 512
                        return xk[k][q][:, off:off + size]

                    def v_proj(g):
                        wvt = []
                        for k in range(KT):
                            w = wvp.tile([128, 512], F16, tag="wv",
                                         name=f"wv{g}_{k}")
                            nc.sync.dma_start(
                                out=w,
                                in_=wvT[k * 128:(k + 1) * 128,
                                        g * 512:(g + 1) * 512])
                            wvt.append(w)
                        for m in range(KT):
                            ps = pp.tile([128, 512], F32, tag="ps1",
                                         name=f"psv{g}_{m}")
                            for k in range(KT):
                                nc.tensor.matmul(
                                    ps, xslice(k, m * 128, 128), wvt[k],
                                    start=(k == 0), stop=(k == KT - 1))
                            vsb = v4p.tile([128, 512], F16, tag="v4",
                                           name=f"v4_{g}_{m}")
                            nc.vector.tensor_copy(out=vsb, in_=ps)
                            v4[g][m] = vsb

                    def qk_weights(h):
                        wts = {}
                        for wT, pname in ((wqT, "q"), (wkT, "k")):
                            wall = wp.tile([128, HID], F16, tag="w",
                                           name=f"w{pname}{h}")
                            nc.sync.dma_start(
                                out=wall,
                                in_=wT[:, h * HID:(h + 1) * HID])
                            wts[pname] = [wall[:, k * 128:(k + 1) * 128]
                                          for k in range(KT)]
                        return wts

                    def qk_proj(h, wts=None):
                        if wts is None:
                            wts = qk_weights(h)
                        for dst, bias_sb, pname in (
                            (qTd, bq_sb, "q"),
                            (kTd, bk_sb, "k"),
                        ):
                            wt = wts[pname]
                            for n in range(NB):
                                ps = pp.tile([128, 512], F32, tag="ps1",
                                             name=f"ps{pname}{h}_{n}")
                                for k in range(KT):
                                    nc.tensor.matmul(
                                        ps, wt[k], xslice(k, n * 512, 512),
                                        start=(k == 0), stop=(k == KT - 1))
                                osb = sp.tile([128, 512], F16, tag="projout",
                                              name=f"o{pname}{h}_{n}")
                                nc.vector.tensor_scalar_add(
                                    osb, ps, bias_sb[:, h:h + 1])
                                nc.sync.dma_start(
                                    out=dst[h * 128:(h + 1) * 128,
                                            n * 512:(n + 1) * 512], in_=osb)

                    # startup: first-needed tiles first
                    load_xq(0)
                    wts0 = qk_weights(0)
                    load_xq(1)
                    load_xq(2)
                    load_xq(3)
                    qk_proj(0, wts0)
                    qk_proj(1)
                    v_proj(0)
                    qk_proj(2)
                    attention(0)
                    qk_proj(3)
                    attention(1)
                    v_proj(1)
                    qk_proj(4)
                    attention(2)
                    qk_proj(5)
                    attention(3)
                    qk_proj(6)
                    attention(4)
                    qk_proj(7)
                    attention(5)

                # phase-1 pools closed: xk/weights SBUF and proj PSUM free.
                # phase 4 opens here so its DMAs/accumulation overlap the
                # attention tail (heads 6,7).
                korder = []
                for h in range(HH):
                    korder += [(h, 0, h), (h, 128, 8 + h)]
                with contextlib.ExitStack() as p4stack:
                    ec4 = p4stack.enter_context
                    wop = ec4(tc.tile_pool(name="p4wo", bufs=32))
                    ctp = ec4(tc.tile_pool(name="p4ct", bufs=16))
                    op_ = ec4(tc.tile_pool(name="p4o", bufs=2))
                    bp4 = ec4(tc.tile_pool(name="p4b", bufs=1))
                    pp4 = ec4(tc.tile_pool(name="ps4", bufs=2, space="PSUM"))
                    attention(6)
                    qk7 = qk_fetch(7)

                    # early ct strips (heads 0-5 fully exchanged by now)
                    ct = [None] * KT

                    def load_strip(ki):
                        c, off, gk = korder[ki]
                        t = ctp.tile([128, S], F16, tag="ct", name=f"ct{gk}")
                        nc.sync.dma_start(
                            out=t[:, 0:SA], in_=recv_a[c][off:off + 128, :])
                        nc.sync.dma_start(
                            out=t[:, SA:S], in_=recv_b[c][off:off + 128, :])
                        ct[ki] = t

                    for ki in range(14):
                        load_strip(ki)

                    bo_bc = bp4.tile([128, HHID], F16, tag="bo_bc")
                    for n in range(HHID // 512):
                        bps = pp4.tile([128, 512], F32, tag="ps4",
                                       name=f"bps{n}")
                        nc.tensor.matmul(bps, ones_row,
                                         bo_sb[:, n * 512:(n + 1) * 512],
                                         start=True, stop=True)
                        nc.vector.tensor_copy(
                            out=bo_bc[:, n * 512:(n + 1) * 512], in_=bps)
                    wo = {}
                    for ki, (c, off, gk) in enumerate(korder):
                        for n in range(HHID // 512):
                            w = wop.tile([128, 512], F16, tag="wo",
                                         name=f"wo{gk}_{n}")
                            nc.sync.dma_start(
                                out=w,
                                in_=woT[gk * 128:(gk + 1) * 128,
                                        n * 512:(n + 1) * 512])
                            wo[(ki, n)] = w

                    attention(7, qk7)

                    for ki in range(14, KT):
                        load_strip(ki)

                    for m in range(S // 128):
                        osb = op_.tile([128, HHID], F32, tag="osb",
                                       name=f"osb{m}")
                        for n in range(HHID // 512):
                            ps = pp4.tile([128, 512], F32, tag="ps4",
                                          name=f"ps4_{m}_{n}")
                            for ki in range(KT):
                                nc.tensor.matmul(
                                    ps, ct[ki][:, m * 128:(m + 1) * 128],
                                    wo[(ki, n)],
                                    start=(ki == 0), stop=(ki == KT - 1))
                            nc.vector.tensor_add(
                                osb[:, n * 512:(n + 1) * 512], ps,
                                bo_bc[:, n * 512:(n + 1) * 512])
                        nc.sync.dma_start(out=out[m * 128:(m + 1) * 128, :],
                                          in_=osb)

    nc.compile()
    return nc


def _get_nc():
    if "nc" not in _CACHED:
        _CACHED["nc"] = _build_program()
    return _CACHED["nc"]


def _make_masks():
    i = np.arange(128)[:, None]
    j = np.arange(1024)[None, :]
    m4 = [((j - i) >= 128 * r).astype(np.float16) for r in range(4)]
    # paired: [mask0 | mask1], [mask2 | mask3] over 512-col halves
    p0 = np.concatenate([m4[0][:, :512], m4[1][:, :512]], axis=1)
    p1 = np.concatenate([m4[2][:, :512], m4[3][:, :512]], axis=1)
    return np.stack([p0, p1], axis=0)


def _make_in_maps(inputs):
    x = np.ascontiguousarray(np.asarray(inputs["x"], dtype=np.float32))
    Wq = np.asarray(inputs["Wq"], dtype=np.float32)
    Wk = np.asarray(inputs["Wk"], dtype=np.float32)
    Wv = np.asarray(inputs["Wv"], dtype=np.float32)
    Wo = np.asarray(inputs["Wo"], dtype=np.float32)
    bq = np.asarray(inputs["bq"], dtype=np.float32)
    bk = np.asarray(inputs["bk"], dtype=np.float32)
    bv = np.asarray(inputs["bv"], dtype=np.float32)
    bo = np.asarray(inputs["bo"], dtype=np.float32)

    bo_eff = bo + Wo @ bv
    masks = _make_masks()
    WqT = np.ascontiguousarray(Wq.T)
    WkT = np.ascontiguousarray(Wk.T)
    WvT = np.ascontiguousarray(Wv.T)
    WoT = np.ascontiguousarray(Wo.T)

    def pack_qk(WT, sl):
        # [p, h*2048 + k*128 + d] = WT[k*128+p, sl][*, h*128+d]
        A = WT[:, sl]
        return np.ascontiguousarray(
            A.reshape(KT, 128, HH, 128).transpose(1, 2, 0, 3)
            .reshape(128, HH * HID)).astype(np.float16)

    in_maps = []
    for c in range(N_CORES):
        b, hf = c // 2, c % 2
        sl = slice(hf * HHID, (hf + 1) * HHID)
        in_maps.append({
            "xT": np.ascontiguousarray(x[b].T).astype(np.float16),
            "wqT": pack_qk(WqT, sl),
            "wkT": pack_qk(WkT, sl),
            "wvT": np.ascontiguousarray(WvT[:, sl]).astype(np.float16),
            "woT": np.ascontiguousarray(WoT[:, sl]).astype(np.float16),
            "bq": np.ascontiguousarray(bq[sl].reshape(HH, 128).T),
            "bk": np.ascontiguousarray(bk[sl].reshape(HH, 128).T),
            "bo": bo_eff[sl].reshape(1, HHID).astype(np.float16),
            "masks": masks,
        })
    return in_maps


def kernel(**inputs):
    from concourse.bass_utils import run_bass_kernel_spmd

    in_maps = _make_in_maps(inputs)
    nc = _get_nc()
    res = run_bass_kernel_spmd(nc, in_maps, list(range(N_CORES)))

    out = np.empty((B, S, HID), dtype=np.float32)
    for c in range(N_CORES):
        b, hf = c // 2, c % 2
        out[b, :, hf * HHID:(hf + 1) * HHID] = res.results[c]["out"]
    return out
